# revision 1
# baseline (speedup 1.0000x reference)
"""Trainium2 Bass kernel for ChannelAttentionModel (segment avg/max -> tiny MLP ->
sigmoid gate -> per-point scale), SPMD across 8 NeuronCores.

Sharding: batch_ids is sorted with B=16 segments; core k owns batches 2k and
2k+1 (whole batches per device). Each batch range is padded to a fixed R points
by replicating the first row of the batch (max-safe); the extra rows' sum
contribution is subtracted via a host-computed correction term.
"""

import sys

for _p in ("/opt/trn_rl_repo", "/root/.axon_site/_ro/trn_rl_repo"):
    if _p not in sys.path:
        sys.path.append(_p)

import numpy as np

import concourse.bacc as bacc
import concourse.tile as tile
from concourse import bass, mybir
from concourse.bass_utils import run_bass_kernel_spmd
from concourse.masks import make_identity

NCORES = 8
B = 16
C = 64
H = 32
RPC = 2  # batch ranges per core
TP = 4096  # points per tile
FA = TP // 128  # free-dim point groups per partition (32)
F = FA * C  # free elems per partition per tile (2048)
DT = mybir.dt.float32


def build_nc(R: int, reps: int = 1, loop_reps: int = 1,
             do_add: bool = True, do_max: bool = True, do_mul: bool = True,
             skip_stats: bool = False, dma_engines: int = 1,
             do_phase1: bool = True, do_phase2: bool = True,
             chunk_tiles: int = 2, xbufs: int = 6, style: str = 'pe',
             max_gps_every: int = 0, inplace: bool = True,
             out_scalar: bool = False):
    NT = R // TP
    nc = bacc.Bacc("TRN2", target_bir_lowering=False, debug=False,
                   num_devices=NCORES, enable_asserts=False)

    xs = nc.dram_tensor("xs", [RPC, R, C], DT, kind="ExternalInput")
    corrt = nc.dram_tensor("corrt", [C, RPC], DT, kind="ExternalInput")
    invct = nc.dram_tensor("invct", [C, RPC], DT, kind="ExternalInput")
    w1t = nc.dram_tensor("w1t", [C, H], DT, kind="ExternalInput")
    b1c = nc.dram_tensor("b1c", [H, 1], DT, kind="ExternalInput")
    w2t = nc.dram_tensor("w2t", [H, C], DT, kind="ExternalInput")
    b2x2 = nc.dram_tensor("b2x2", [C, 1], DT, kind="ExternalInput")
    out = nc.dram_tensor("out", [RPC, R, C], DT, kind="ExternalOutput")

    def dram_chunk_ap(handle, r, off, npts):
        return handle.ap()[r, off:off + npts, :].rearrange(
            "(p a) c -> p (a c)", p=128)

    # chunk each range into large DMA transfers
    chunks = []
    off = 0
    while off < R:
        npts = min(chunk_tiles * TP, R - off)
        chunks.append((off, npts))
        off += npts

    with tile.TileContext(nc) as tc:
        with (
            tc.tile_pool(name="const", bufs=1) as const,
            tc.tile_pool(name="xpool", bufs=xbufs) as xpool,
            tc.tile_pool(name="accs", bufs=1) as accs,
            tc.tile_pool(name="stat", bufs=4) as stat,
            tc.tile_pool(name="small", bufs=1) as small,
            tc.tile_pool(name="psum_t", bufs=1, space="PSUM") as psum_t,
            tc.tile_pool(name="psum_w", bufs=1, space="PSUM") as psum_w,
        ):
            # constants
            ident = const.tile([128, 128], DT)
            make_identity(nc, ident[:])
            ones_row = const.tile([1, 128], DT)
            nc.vector.memset(ones_row[:], 1.0)
            ones_bf = const.tile([128, 1], mybir.dt.bfloat16)
            nc.vector.memset(ones_bf[:], 1.0)
            corrt_sb = const.tile([C, RPC], DT)
            nc.sync.dma_start(out=corrt_sb[:], in_=corrt.ap())
            invct_sb = const.tile([C, RPC], DT)
            nc.sync.dma_start(out=invct_sb[:], in_=invct.ap())
            w1t_sb = const.tile([C, H], DT)
            nc.sync.dma_start(out=w1t_sb[:], in_=w1t.ap())
            b1c_sb = const.tile([H, 1], DT)
            nc.sync.dma_start(out=b1c_sb[:], in_=b1c.ap())
            w2t_sb = const.tile([H, C], DT)
            nc.sync.dma_start(out=w2t_sb[:], in_=w2t.ap())
            b2x2_sb = const.tile([C, 1], DT)
            nc.sync.dma_start(out=b2x2_sb[:], in_=b2x2.ap())

            import contextlib
            loop_cm = tc.For_i(0, loop_reps, 1) if loop_reps > 1 else contextlib.nullcontext()
            with loop_cm:
                for rep in range(reps):
                    # phase 1: per-range running sum / max over streamed tiles
                    rhs4 = small.tile([C, 2 * RPC], DT)  # cols: avg0, avg1, mx0, mx1
                    FCMAX = chunk_tiles * F
                    for r in range(RPC):
                        m_acc = accs.tile([128, FCMAX], DT, tag="m_acc")
                        nc.vector.memset(m_acc[:], -1e30)
                        if style == 'pe':
                            ps_s = psum_t.tile([1, 512], DT, tag=f"ps_s{r}")
                            nmm = sum(-(-npts * C // 128) // 512 for _, npts in chunks)
                            mmi = 0
                        else:
                            s_acc = accs.tile([128, FCMAX], DT, tag="s_acc")
                            nc.vector.memset(s_acc[:], 0.0)
                        for ci, (off, npts) in enumerate(chunks):
                            if not do_phase1:
                                break
                            fc = npts * C // 128
                            xt = xpool.tile([128, FCMAX], DT, tag="xt")
                            eng = [nc.sync, nc.scalar, nc.gpsimd][ci % dma_engines]
                            eng.dma_start(out=xt[:, :fc],
                                          in_=dram_chunk_ap(xs, r, off, npts))
                            if do_max:
                                meng = (nc.gpsimd if (max_gps_every and
                                        ci % max_gps_every == max_gps_every - 1)
                                        else nc.vector)
                                meng.tensor_max(m_acc[:, :fc], m_acc[:, :fc],
                                                xt[:, :fc])
                            if do_add and style == 'pe':
                                xb = stat.tile([128, FCMAX], mybir.dt.bfloat16,
                                               tag="xb")
                                nc.scalar.copy(out=xb[:, :fc], in_=xt[:, :fc])
                                for j in range(fc // 512):
                                    nc.tensor.matmul(
                                        out=ps_s[:], lhsT=ones_bf[:],
                                        rhs=xb[:, j * 512:(j + 1) * 512],
                                        start=(mmi == 0), stop=(mmi == nmm - 1))
                                    mmi += 1
                            elif do_add:
                                for j in range(fc // F):
                                    nc.vector.tensor_add(
                                        s_acc[:, j * F:(j + 1) * F],
                                        s_acc[:, j * F:(j + 1) * F],
                                        xt[:, j * F:(j + 1) * F])

                        # fold sums
                        sum_col = small.tile([C, 1], DT, tag=f"sum_col{r}")
                        if style == 'pe':
                            sum_row = small.tile([1, C], DT, tag=f"sum_row{r}")
                            nc.vector.reduce_sum(
                                out=sum_row[:],
                                in_=ps_s[:].rearrange("p (a c) -> p c a", c=C),
                                axis=mybir.AxisListType.X)
                            sc_ps = psum_w.tile([C, 1], DT, tag="sc")
                            nc.tensor.transpose(out=sc_ps[:], in_=sum_row[:],
                                                identity=ident[:1, :1])
                            nc.vector.tensor_copy(sum_col[:], sc_ps[:])
                        else:
                            s64 = small.tile([128, C], DT, tag="s64")
                            nc.vector.reduce_sum(
                                out=s64[:],
                                in_=s_acc[:].rearrange("p (a c) -> p c a", c=C),
                                axis=mybir.AxisListType.X)
                            srow_t = psum_t.tile([C, 128], DT, tag="tr")
                            nc.tensor.transpose(out=srow_t[:], in_=s64[:],
                                                identity=ident[:])
                            nc.vector.reduce_sum(out=sum_col[:], in_=srow_t[:],
                                                 axis=mybir.AxisListType.X)

                        # fold max: free fold then partition fold
                        m64 = small.tile([128, C], DT, tag="m64")
                        nc.vector.reduce_max(
                            out=m64[:],
                            in_=m_acc[:].rearrange("p (a c) -> p c a", c=C),
                            axis=mybir.AxisListType.X)
                        mrow_t = psum_t.tile([C, 128], DT, tag="tr")
                        nc.tensor.transpose(out=mrow_t[:], in_=m64[:], identity=ident[:])
                        nc.vector.reduce_max(out=rhs4[:, RPC + r:RPC + r + 1], in_=mrow_t[:],
                                             axis=mybir.AxisListType.X)

                        # avg = (sum - corr) * invc
                        nc.vector.tensor_sub(sum_col[:], sum_col[:], corrt_sb[:, r:r + 1])
                        nc.vector.tensor_mul(rhs4[:, r:r + 1], sum_col[:],
                                             invct_sb[:, r:r + 1])

                    # tiny MLP: att = sigmoid(mlp(avg) + mlp(mx)); scale = 1 + att
                    h_ps = psum_w.tile([H, 2 * RPC], DT, tag="mm")
                    nc.tensor.matmul(out=h_ps[:], lhsT=w1t_sb[:], rhs=rhs4[:],
                                     start=True, stop=True)
                    h_sb = small.tile([H, 2 * RPC], DT)
                    nc.scalar.activation(out=h_sb[:], in_=h_ps[:],
                                         func=mybir.ActivationFunctionType.Relu,
                                         bias=b1c_sb[:])
                    z_ps = psum_w.tile([C, 2 * RPC], DT, tag="mm")
                    nc.tensor.matmul(out=z_ps[:], lhsT=w2t_sb[:], rhs=h_sb[:],
                                     start=True, stop=True)
                    z_sb = small.tile([C, 2 * RPC], DT)
                    nc.vector.tensor_copy(z_sb[:], z_ps[:])
                    zsum = small.tile([C, RPC], DT)
                    nc.vector.tensor_add(zsum[:], z_sb[:, 0:RPC], z_sb[:, RPC:2 * RPC])
                    scale_t = small.tile([C, RPC], DT)
                    nc.scalar.activation(out=scale_t[:], in_=zsum[:],
                                         func=mybir.ActivationFunctionType.Sigmoid,
                                         bias=b2x2_sb[:])
                    nc.vector.tensor_scalar_add(scale_t[:], scale_t[:], 1.0)

                    # broadcast each range's scale column to [128, C]
                    mults = []
                    for r in range(RPC):
                        row_ps = psum_w.tile([1, C], DT, tag="row")
                        nc.tensor.transpose(out=row_ps[:], in_=scale_t[:, r:r + 1],
                                            identity=ident[:C, :C])
                        row_sb = small.tile([1, C], DT, tag=f"row_sb{r}")
                        nc.vector.tensor_copy(row_sb[:], row_ps[:])
                        bcast_ps = psum_w.tile([128, C], DT, tag="bc")
                        nc.tensor.matmul(out=bcast_ps[:], lhsT=ones_row[:], rhs=row_sb[:],
                                         start=True, stop=True)
                        mult = accs.tile([128, C], DT, tag=f"mult{r}")
                        nc.vector.tensor_copy(mult[:], bcast_ps[:])
                        mults.append(mult)

                    # phase 2: out = x * scale[batch]
                    for r in range(RPC):
                        for ci, (off, npts) in enumerate(chunks):
                            if not do_phase2:
                                break
                            fa = npts // 128
                            mult_bc = mults[r][:].unsqueeze(1).to_broadcast(
                                [128, fa, C])
                            xt = xpool.tile([128, chunk_tiles * FA, C], DT, tag="xt")
                            eng_i = [nc.sync, nc.scalar, nc.gpsimd][ci % dma_engines]
                            if out_scalar:
                                eng_o = nc.scalar
                            else:
                                eng_o = [nc.sync, nc.scalar, nc.gpsimd][
                                    (ci + 1) % dma_engines]
                            eng_i.dma_start(out=xt[:, :fa, :],
                                            in_=dram_chunk_ap(xs, r, off, npts))
                            yt = xt if inplace else xpool.tile(
                                [128, chunk_tiles * FA, C], DT, tag="yt")
                            if do_mul:
                                nc.vector.tensor_mul(yt[:, :fa, :], xt[:, :fa, :],
                                                     mult_bc)
                            eng_o.dma_start(out=dram_chunk_ap(out, r, off, npts),
                                            in_=yt[:, :fa, :])

    nc.compile()
    return nc


_CACHE: dict[int, object] = {}


def kernel(x, batch_ids, W1, b1, W2, b2):
    x = np.ascontiguousarray(np.asarray(x, dtype=np.float32))
    batch_ids = np.asarray(batch_ids, dtype=np.int32)
    W1 = np.asarray(W1, dtype=np.float32)
    b1 = np.asarray(b1, dtype=np.float32)
    W2 = np.asarray(W2, dtype=np.float32)
    b2 = np.asarray(b2, dtype=np.float32)

    N = x.shape[0]
    bounds = np.searchsorted(batch_ids, np.arange(B + 1), side="left")
    counts = np.diff(bounds)
    R = max(TP, int(-(-counts.max() // TP)) * TP)

    nc = _CACHE.get(R)
    if nc is None:
        nc = _CACHE[R] = build_nc(R)

    xp = np.empty((NCORES, RPC, R, C), np.float32)
    corrt = np.zeros((NCORES, C, RPC), np.float32)
    invct = np.zeros((NCORES, C, RPC), np.float32)
    for b in range(B):
        core, r = divmod(b, RPC)
        s, e = int(bounds[b]), int(bounds[b + 1])
        n = e - s
        xp[core, r, :n] = x[s:e]
        pad = x[s] if n > 0 else np.zeros(C, np.float32)
        xp[core, r, n:] = pad
        corrt[core, :, r] = np.float64(R - n) * pad.astype(np.float64)
        invct[core, :, r] = 1.0 / max(n, 1)

    w1t = np.ascontiguousarray(W1.T)  # [C, H]
    b1c = np.ascontiguousarray(b1.reshape(H, 1))
    w2t = np.ascontiguousarray(W2.T)  # [H, C]
    b2x2 = np.ascontiguousarray((2.0 * b2).reshape(C, 1))

    in_maps = [
        {
            "xs": xp[core],
            "corrt": np.ascontiguousarray(corrt[core]),
            "invct": np.ascontiguousarray(invct[core]),
            "w1t": w1t,
            "b1c": b1c,
            "w2t": w2t,
            "b2x2": b2x2,
        }
        for core in range(NCORES)
    ]

    res = run_bass_kernel_spmd(nc, in_maps, core_ids=list(range(NCORES)))

    out = np.empty((N, C), np.float32)
    for b in range(B):
        core, r = divmod(b, RPC)
        s, e = int(bounds[b]), int(bounds[b + 1])
        out[s:e] = res.results[core]["out"][r, : e - s]
    return out



# revision 2
# speedup vs baseline: 2.3053x; 2.3053x over previous
"""Trainium2 Bass kernel for ChannelAttentionModel (segment avg/max -> tiny MLP ->
sigmoid gate -> per-point scale), SPMD across 8 NeuronCores.

Sharding: batch_ids is sorted with B=16 segments; core k owns batches 2k and
2k+1 (whole batches per device). Each batch range is padded to a fixed R points
by replicating the first row of the batch (max-safe); the extra rows' sum
contribution is subtracted via a host-computed correction term.

Memory optimization: x is uploaded as fp16 (halves read bytes) and the output
is written as fp16 (halves write bytes; host converts back to f32). Range 0's
fp16 tiles are kept resident in SBUF during phase 1 so phase 2 re-reads only
range 1. Per-core HBM traffic: 16.25 (r0 read) + 16.25 (r1 read) + 16.25
(r1 re-read) + 32.5 (out write) = 81.25 MB, vs 195 MB for the f32 two-pass
version. fp16 end-to-end error ~1.5e-3 elementwise (gate is 2e-2).
"""

import sys

for _p in ("/opt/trn_rl_repo", "/root/.axon_site/_ro/trn_rl_repo"):
    if _p not in sys.path:
        sys.path.append(_p)

import numpy as np

import concourse.bacc as bacc
import concourse.tile as tile
from concourse import bass, mybir
from concourse.bass_utils import run_bass_kernel_spmd
from concourse.masks import make_identity

NCORES = 8
B = 16
C = 64
H = 32
RPC = 2  # batch ranges per core
TP = 4096  # points per tile
DT = mybir.dt.float32
DT16 = mybir.dt.float16


def build_nc(R: int, chunk_tiles: int = 2, stash: bool = True,
             xbufs: int = 3, p2bufs: int = 3):
    nc = bacc.Bacc("TRN2", target_bir_lowering=False, debug=False,
                   num_devices=NCORES, enable_asserts=False)

    xs = nc.dram_tensor("xs", [RPC, R, C], DT16, kind="ExternalInput")
    corrt = nc.dram_tensor("corrt", [C, RPC], DT, kind="ExternalInput")
    invct = nc.dram_tensor("invct", [C, RPC], DT, kind="ExternalInput")
    w1t = nc.dram_tensor("w1t", [C, H], DT, kind="ExternalInput")
    b1c = nc.dram_tensor("b1c", [H, 1], DT, kind="ExternalInput")
    w2t = nc.dram_tensor("w2t", [H, C], DT, kind="ExternalInput")
    b2x2 = nc.dram_tensor("b2x2", [C, 1], DT, kind="ExternalInput")
    out = nc.dram_tensor("out", [RPC, R, C], DT16, kind="ExternalOutput")

    def dram_chunk_ap(handle, r, off, npts):
        return handle.ap()[r, off:off + npts, :].rearrange(
            "(p a) c -> p (a c)", p=128)

    chunks = []
    off = 0
    while off < R:
        npts = min(chunk_tiles * TP, R - off)
        chunks.append((off, npts))
        off += npts
    n_chunks = len(chunks)
    FCMAX = chunk_tiles * TP * C // 128  # free elems per partition per chunk

    with tile.TileContext(nc) as tc:
        with (
            tc.tile_pool(name="const", bufs=1) as const,
            tc.tile_pool(name="stashp", bufs=(n_chunks if stash else 1)) as stashp,
            tc.tile_pool(name="xpool", bufs=xbufs) as xpool,
            tc.tile_pool(name="p2pool", bufs=p2bufs) as p2pool,
            tc.tile_pool(name="accs", bufs=2) as accs,
            tc.tile_pool(name="small", bufs=4) as small,
            tc.tile_pool(name="psum_t", bufs=1, space="PSUM") as psum_t,
            tc.tile_pool(name="psum_w", bufs=1, space="PSUM") as psum_w,
        ):
            # constants
            ident = const.tile([128, 128], DT)
            make_identity(nc, ident[:])
            ones_row = const.tile([1, 128], DT)
            nc.vector.memset(ones_row[:], 1.0)
            ones16 = const.tile([128, 1], DT16)
            nc.vector.memset(ones16[:], 1.0)
            corrt_sb = const.tile([C, RPC], DT)
            nc.sync.dma_start(out=corrt_sb[:], in_=corrt.ap())
            invct_sb = const.tile([C, RPC], DT)
            nc.sync.dma_start(out=invct_sb[:], in_=invct.ap())
            w1t_sb = const.tile([C, H], DT)
            nc.sync.dma_start(out=w1t_sb[:], in_=w1t.ap())
            b1c_sb = const.tile([H, 1], DT)
            nc.sync.dma_start(out=b1c_sb[:], in_=b1c.ap())
            w2t_sb = const.tile([H, C], DT)
            nc.sync.dma_start(out=w2t_sb[:], in_=w2t.ap())
            b2x2_sb = const.tile([C, 1], DT)
            nc.sync.dma_start(out=b2x2_sb[:], in_=b2x2.ap())

            # phase 1: per-range running max (DVE) + sum (PE) over fp16 tiles
            rhs4 = small.tile([C, 2 * RPC], DT)  # cols: avg0, avg1, mx0, mx1
            stash_tiles = []
            for r in range(RPC):
                m_acc = accs.tile([128, FCMAX], DT16, tag="m_acc")
                nc.vector.memset(m_acc[:], -65504.0)
                ps_s = psum_t.tile([1, 512], DT, tag=f"ps_s{r}")
                nmm = sum((npts * C // 128) // 512 for _, npts in chunks)
                mmi = 0
                for ci, (off, npts) in enumerate(chunks):
                    fc = npts * C // 128
                    if stash and r == 0:
                        xt = stashp.tile([128, FCMAX], DT16, tag="stash")
                        stash_tiles.append(xt)
                    else:
                        xt = xpool.tile([128, FCMAX], DT16, tag="xt")
                    nc.sync.dma_start(out=xt[:, :fc],
                                      in_=dram_chunk_ap(xs, r, off, npts))
                    nc.vector.tensor_max(m_acc[:, :fc], m_acc[:, :fc],
                                         xt[:, :fc])
                    for j in range(fc // 512):
                        nc.tensor.matmul(
                            out=ps_s[:], lhsT=ones16[:],
                            rhs=xt[:, j * 512:(j + 1) * 512],
                            start=(mmi == 0), stop=(mmi == nmm - 1))
                        mmi += 1

                # fold sums: ps_s free layout is (a8, c); reduce over a8
                sum_col = small.tile([C, 1], DT, tag=f"sum_col{r}")
                sum_row = small.tile([1, C], DT, tag=f"sum_row{r}")
                nc.vector.reduce_sum(
                    out=sum_row[:],
                    in_=ps_s[:].rearrange("p (a c) -> p c a", c=C),
                    axis=mybir.AxisListType.X)
                sc_ps = psum_w.tile([C, 1], DT, tag="sc")
                nc.tensor.transpose(out=sc_ps[:], in_=sum_row[:],
                                    identity=ident[:1, :1])
                nc.vector.tensor_copy(sum_col[:], sc_ps[:])

                # fold max: free fold then partition fold
                m64 = small.tile([128, C], DT, tag="m64")
                nc.vector.reduce_max(
                    out=m64[:],
                    in_=m_acc[:].rearrange("p (a c) -> p c a", c=C),
                    axis=mybir.AxisListType.X)
                mrow_t = psum_t.tile([C, 128], DT, tag="tr")
                nc.tensor.transpose(out=mrow_t[:], in_=m64[:], identity=ident[:])
                nc.vector.reduce_max(out=rhs4[:, RPC + r:RPC + r + 1],
                                     in_=mrow_t[:], axis=mybir.AxisListType.X)

                # avg = (sum - corr) * invc
                nc.vector.tensor_sub(sum_col[:], sum_col[:], corrt_sb[:, r:r + 1])
                nc.vector.tensor_mul(rhs4[:, r:r + 1], sum_col[:],
                                     invct_sb[:, r:r + 1])

            # tiny MLP: att = sigmoid(mlp(avg) + mlp(mx)); scale = 1 + att
            h_ps = psum_w.tile([H, 2 * RPC], DT, tag="mm")
            nc.tensor.matmul(out=h_ps[:], lhsT=w1t_sb[:], rhs=rhs4[:],
                             start=True, stop=True)
            h_sb = small.tile([H, 2 * RPC], DT)
            nc.scalar.activation(out=h_sb[:], in_=h_ps[:],
                                 func=mybir.ActivationFunctionType.Relu,
                                 bias=b1c_sb[:])
            z_ps = psum_w.tile([C, 2 * RPC], DT, tag="mm")
            nc.tensor.matmul(out=z_ps[:], lhsT=w2t_sb[:], rhs=h_sb[:],
                             start=True, stop=True)
            z_sb = small.tile([C, 2 * RPC], DT)
            nc.vector.tensor_copy(z_sb[:], z_ps[:])
            zsum = small.tile([C, RPC], DT)
            nc.vector.tensor_add(zsum[:], z_sb[:, 0:RPC], z_sb[:, RPC:2 * RPC])
            scale_t = small.tile([C, RPC], DT)
            nc.scalar.activation(out=scale_t[:], in_=zsum[:],
                                 func=mybir.ActivationFunctionType.Sigmoid,
                                 bias=b2x2_sb[:])
            nc.vector.tensor_scalar_add(scale_t[:], scale_t[:], 1.0)

            # broadcast each range's scale column to [128, C] fp16
            mults = []
            for r in range(RPC):
                row_ps = psum_w.tile([1, C], DT, tag="row")
                nc.tensor.transpose(out=row_ps[:], in_=scale_t[:, r:r + 1],
                                    identity=ident[:C, :C])
                row_sb = small.tile([1, C], DT, tag=f"row_sb{r}")
                nc.vector.tensor_copy(row_sb[:], row_ps[:])
                bcast_ps = psum_w.tile([128, C], DT, tag="bc")
                nc.tensor.matmul(out=bcast_ps[:], lhsT=ones_row[:], rhs=row_sb[:],
                                 start=True, stop=True)
                mult = accs.tile([128, C], DT16, tag=f"mult{r}")
                nc.vector.tensor_copy(mult[:], bcast_ps[:])
                mults.append(mult)

            # phase 2: out = x * scale[batch]; r0 from stash (no re-read)
            for r in range(RPC):
                for ci, (off, npts) in enumerate(chunks):
                    fa = npts // 128
                    if stash and r == 0:
                        xt = stash_tiles[ci]
                    else:
                        xt = p2pool.tile([128, FCMAX], DT16, tag="p2")
                        nc.sync.dma_start(out=xt[:, :fa * C],
                                          in_=dram_chunk_ap(xs, r, off, npts))
                    xv = xt[:, :fa * C].rearrange("p (a c) -> p a c", c=C)
                    mult_bc = mults[r][:].unsqueeze(1).to_broadcast([128, fa, C])
                    nc.vector.tensor_mul(xv, xv, mult_bc)
                    nc.scalar.dma_start(out=dram_chunk_ap(out, r, off, npts),
                                        in_=xt[:, :fa * C])

    nc.compile()
    return nc


_CACHE: dict[tuple, object] = {}


def kernel(x, batch_ids, W1, b1, W2, b2):
    x = np.ascontiguousarray(np.asarray(x, dtype=np.float32))
    batch_ids = np.asarray(batch_ids, dtype=np.int32)
    W1 = np.asarray(W1, dtype=np.float32)
    b1 = np.asarray(b1, dtype=np.float32)
    W2 = np.asarray(W2, dtype=np.float32)
    b2 = np.asarray(b2, dtype=np.float32)

    N = x.shape[0]
    bounds = np.searchsorted(batch_ids, np.arange(B + 1), side="left")
    counts = np.diff(bounds)
    R = max(TP, int(-(-counts.max() // TP)) * TP)

    nc = _CACHE.get(R)
    if nc is None:
        nc = _CACHE[R] = build_nc(R)

    xh = x.astype(np.float16)
    xp = np.empty((NCORES, RPC, R, C), np.float16)
    corrt = np.zeros((NCORES, C, RPC), np.float32)
    invct = np.zeros((NCORES, C, RPC), np.float32)
    for b in range(B):
        core, r = divmod(b, RPC)
        s, e = int(bounds[b]), int(bounds[b + 1])
        n = e - s
        xp[core, r, :n] = xh[s:e]
        pad = xh[s] if n > 0 else np.zeros(C, np.float16)
        xp[core, r, n:] = pad
        corrt[core, :, r] = np.float64(R - n) * pad.astype(np.float64)
        invct[core, :, r] = 1.0 / max(n, 1)

    w1t = np.ascontiguousarray(W1.T)  # [C, H]
    b1c = np.ascontiguousarray(b1.reshape(H, 1))
    w2t = np.ascontiguousarray(W2.T)  # [H, C]
    b2x2 = np.ascontiguousarray((2.0 * b2).reshape(C, 1))

    in_maps = [
        {
            "xs": xp[core],
            "corrt": np.ascontiguousarray(corrt[core]),
            "invct": np.ascontiguousarray(invct[core]),
            "w1t": w1t,
            "b1c": b1c,
            "w2t": w2t,
            "b2x2": b2x2,
        }
        for core in range(NCORES)
    ]

    res = run_bass_kernel_spmd(nc, in_maps, core_ids=list(range(NCORES)))

    out = np.empty((N, C), np.float32)
    for b in range(B):
        core, r = divmod(b, RPC)
        s, e = int(bounds[b]), int(bounds[b + 1])
        out[s:e] = res.results[core]["out"][r, : e - s].astype(np.float32)
    return out


# revision 8
# speedup vs baseline: 2.4222x; 1.0507x over previous
"""Trainium2 Bass kernel for ChannelAttentionModel (segment avg/max -> tiny MLP ->
sigmoid gate -> per-point scale), SPMD across 8 NeuronCores.

Sharding: batch_ids is sorted with B=16 segments; core k owns batches 2k and
2k+1 (whole batches per device). Each batch range is padded to a fixed R points
by replicating the first row of the batch (max-safe); the extra rows' sum
contribution is subtracted via a host-computed correction term.

Memory optimization: x is uploaded as fp16 (halves read bytes) and the output
is written as fp16 (halves write bytes; host converts back to f32). Range 0
(and the first few chunks of range 1) stay resident in SBUF through phase 1,
so phase 2 re-reads only the tail of range 1. Per-core HBM traffic ~76 MB vs
195 MB for the f32 two-pass version. fp16 end-to-end error ~1.5e-3
elementwise (gate is 2e-2).

Pipelining: each range runs fold -> MLP -> scale broadcast -> phase 2
immediately, so range 0's output writes (ACT HWDGE ring) overlap range 1's
input reads (SP HWDGE ring). Emission interleaves r1 phase-1 chunks with r0
phase-2 chunks so the in-order DVE queue alternates max/mul.
"""

import sys

for _p in ("/opt/trn_rl_repo", "/root/.axon_site/_ro/trn_rl_repo"):
    if _p not in sys.path:
        sys.path.append(_p)

import numpy as np

import concourse.bacc as bacc
import concourse.tile as tile
from concourse import bass, mybir
from concourse.bass_utils import run_bass_kernel_spmd
from concourse.masks import make_identity

NCORES = 8
B = 16
C = 64
H = 32
RPC = 2  # batch ranges per core
TP = 4096  # points per tile
DT = mybir.dt.float32
DT16 = mybir.dt.float16


def build_nc(R: int, chunk_tiles: int = 2, n_stash_r1: int = 4,
             sbufs: int = 4):
    nc = bacc.Bacc("TRN2", target_bir_lowering=False, debug=False,
                   num_devices=NCORES, enable_asserts=False)

    xs = nc.dram_tensor("xs", [RPC, R, C], DT16, kind="ExternalInput")
    corrt = nc.dram_tensor("corrt", [C, RPC], DT, kind="ExternalInput")
    invct = nc.dram_tensor("invct", [C, RPC], DT, kind="ExternalInput")
    w1t = nc.dram_tensor("w1t", [C, H], DT, kind="ExternalInput")
    b1c = nc.dram_tensor("b1c", [H, 1], DT, kind="ExternalInput")
    w2t = nc.dram_tensor("w2t", [H, C], DT, kind="ExternalInput")
    b2s = nc.dram_tensor("b2s", [C, 1], DT, kind="ExternalInput")
    out = nc.dram_tensor("out", [RPC, R, C], DT16, kind="ExternalOutput")

    def dram_chunk_ap(handle, r, off, npts):
        return handle.ap()[r, off:off + npts, :].rearrange(
            "(p a) c -> p (a c)", p=128)

    chunks = []
    off = 0
    while off < R:
        npts = min(chunk_tiles * TP, R - off)
        chunks.append((off, npts))
        off += npts
    n_chunks = len(chunks)
    n_stash_r1 = min(n_stash_r1, n_chunks)
    n_stash = n_chunks + n_stash_r1
    FCMAX = chunk_tiles * TP * C // 128  # free elems per partition per chunk

    with tile.TileContext(nc) as tc:
        with (
            tc.tile_pool(name="const", bufs=1) as const,
            tc.tile_pool(name="stashp", bufs=n_stash) as stashp,
            tc.tile_pool(name="xpool", bufs=sbufs) as xpool,
            tc.tile_pool(name="accs", bufs=1) as accs,
            tc.tile_pool(name="small", bufs=1) as small,
            tc.tile_pool(name="psum_t", bufs=1, space="PSUM") as psum_t,
            tc.tile_pool(name="psum_w", bufs=1, space="PSUM") as psum_w,
        ):
            # constants
            ident = const.tile([128, 128], DT)
            make_identity(nc, ident[:])
            ones_row = const.tile([1, 128], DT)
            nc.vector.memset(ones_row[:], 1.0)
            ones16 = const.tile([128, 1], DT16)
            nc.vector.memset(ones16[:], 1.0)
            # const loads go on the ACT HWDGE ring (idle until phase 2) so
            # the first big x read on the SP ring starts immediately
            corrt_sb = const.tile([C, RPC], DT)
            nc.scalar.dma_start(out=corrt_sb[:], in_=corrt.ap())
            invct_sb = const.tile([C, RPC], DT)
            nc.scalar.dma_start(out=invct_sb[:], in_=invct.ap())
            w1t_sb = const.tile([C, H], DT)
            nc.scalar.dma_start(out=w1t_sb[:], in_=w1t.ap())
            b1c_sb = const.tile([H, 1], DT)
            nc.scalar.dma_start(out=b1c_sb[:], in_=b1c.ap())
            w2t_sb = const.tile([H, C], DT)
            nc.scalar.dma_start(out=w2t_sb[:], in_=w2t.ap())
            b2s_sb = const.tile([C, 1], DT)
            nc.scalar.dma_start(out=b2s_sb[:], in_=b2s.ap())

            stash_tiles = {}  # (r, ci) -> tile

            def p1_chunk(r, ci, ps_s, m_acc, mm_state):
                """Emit phase-1 work for one chunk: DMA in, max, PE sum."""
                off, npts = chunks[ci]
                fc = npts * C // 128
                if r == 0 or ci < n_stash_r1:
                    xt = stashp.tile([128, FCMAX], DT16, tag="stash")
                    stash_tiles[(r, ci)] = xt
                else:
                    xt = xpool.tile([128, FCMAX], DT16, tag="xt")
                nc.sync.dma_start(out=xt[:, :fc],
                                  in_=dram_chunk_ap(xs, r, off, npts))
                nc.vector.tensor_max(m_acc[:, :fc], m_acc[:, :fc], xt[:, :fc])
                for j in range(fc // 512):
                    nc.tensor.matmul(
                        out=ps_s[:], lhsT=ones16[:],
                        rhs=xt[:, j * 512:(j + 1) * 512],
                        start=(mm_state[0] == 0),
                        stop=(mm_state[0] == mm_state[1] - 1))
                    mm_state[0] += 1

            def fold_and_mlp(r, ps_s, m_acc):
                """Stats fold + per-range MLP -> fp16 [128, C] scale tile."""
                # fold sums: ps_s free layout is (a8, c); reduce over a8
                sum_col = small.tile([C, 1], DT, tag=f"sum_col{r}")
                sum_row = small.tile([1, C], DT, tag=f"sum_row{r}")
                nc.vector.reduce_sum(
                    out=sum_row[:],
                    in_=ps_s[:].rearrange("p (a c) -> p c a", c=C),
                    axis=mybir.AxisListType.X)
                sc_ps = psum_w.tile([C, 1], DT, tag="sc")
                nc.tensor.transpose(out=sc_ps[:], in_=sum_row[:],
                                    identity=ident[:1, :1])
                nc.vector.tensor_copy(sum_col[:], sc_ps[:])

                # fold max: contiguous binary tree over the free dim
                w = FCMAX
                while w > C:
                    h = w // 2
                    nc.vector.tensor_max(m_acc[:, :h], m_acc[:, :h],
                                         m_acc[:, h:w])
                    w = h
                m64 = small.tile([128, C], DT, tag="m64")
                nc.vector.tensor_copy(m64[:], m_acc[:, :C])
                mrow_t = psum_t.tile([C, 128], DT, tag="tr")
                nc.tensor.transpose(out=mrow_t[:], in_=m64[:],
                                    identity=ident[:])
                rhs2 = small.tile([C, 2], DT, tag=f"rhs2_{r}")
                nc.vector.reduce_max(out=rhs2[:, 1:2], in_=mrow_t[:],
                                     axis=mybir.AxisListType.X)

                # avg = (sum - corr) * invc
                nc.vector.tensor_sub(sum_col[:], sum_col[:],
                                     corrt_sb[:, r:r + 1])
                nc.vector.tensor_mul(rhs2[:, 0:1], sum_col[:],
                                     invct_sb[:, r:r + 1])

                # tiny MLP on [C, 2]: att = sigmoid(mlp(avg) + mlp(mx))
                h_ps = psum_w.tile([H, 2], DT, tag="mm")
                nc.tensor.matmul(out=h_ps[:], lhsT=w1t_sb[:], rhs=rhs2[:],
                                 start=True, stop=True)
                h_sb = small.tile([H, 2], DT, tag=f"h_sb{r}")
                nc.scalar.activation(out=h_sb[:], in_=h_ps[:],
                                     func=mybir.ActivationFunctionType.Relu,
                                     bias=b1c_sb[:])
                z_ps = psum_w.tile([C, 2], DT, tag="mm")
                nc.tensor.matmul(out=z_ps[:], lhsT=w2t_sb[:], rhs=h_sb[:],
                                 start=True, stop=True)
                z_sb = small.tile([C, 2], DT, tag=f"z_sb{r}")
                nc.vector.tensor_copy(z_sb[:], z_ps[:])
                zsum = small.tile([C, 1], DT, tag=f"zsum{r}")
                nc.vector.tensor_add(zsum[:], z_sb[:, 0:1], z_sb[:, 1:2])
                sig = small.tile([C, 1], DT, tag=f"sig{r}")
                nc.scalar.activation(out=sig[:], in_=zsum[:],
                                     func=mybir.ActivationFunctionType.Sigmoid,
                                     bias=b2s_sb[:])

                # broadcast [C,1] -> fp16 [128, C], with the +1.0 folded in
                row_ps = psum_w.tile([1, C], DT, tag="row")
                nc.tensor.transpose(out=row_ps[:], in_=sig[:],
                                    identity=ident[:C, :C])
                row_sb = small.tile([1, C], DT, tag=f"row_sb{r}")
                nc.vector.tensor_copy(row_sb[:], row_ps[:])
                bcast_ps = psum_w.tile([128, C], DT, tag="bc")
                nc.tensor.matmul(out=bcast_ps[:], lhsT=ones_row[:],
                                 rhs=row_sb[:], start=True, stop=True)
                mult = accs.tile([128, C], DT16, tag=f"mult{r}")
                nc.vector.tensor_scalar_add(mult[:], bcast_ps[:], 1.0)
                return mult

            def p2_chunk(r, ci, mult):
                """Emit phase-2 work for one chunk: (DMA in,) mul, DMA out."""
                off, npts = chunks[ci]
                fa = npts // 128
                xt = stash_tiles.get((r, ci))
                if xt is None:
                    xt = xpool.tile([128, FCMAX], DT16, tag="xt")
                    nc.sync.dma_start(out=xt[:, :fa * C],
                                      in_=dram_chunk_ap(xs, r, off, npts))
                xv = xt[:, :fa * C].rearrange("p (a c) -> p a c", c=C)
                mult_bc = mult[:].unsqueeze(1).to_broadcast([128, fa, C])
                nc.vector.tensor_mul(xv, xv, mult_bc)
                nc.scalar.dma_start(out=dram_chunk_ap(out, r, off, npts),
                                    in_=xt[:, :fa * C])

            nmm = sum((npts * C // 128) // 512 for _, npts in chunks)

            # ---- range 0 phase 1 ----
            m_acc0 = accs.tile([128, FCMAX], DT16, tag="m_acc")
            nc.vector.memset(m_acc0[:], -65504.0)
            ps_s0 = psum_t.tile([1, 512], DT, tag="ps_s0")
            st0 = [0, nmm]
            for ci in range(n_chunks):
                p1_chunk(0, ci, ps_s0, m_acc0, st0)
            mult0 = fold_and_mlp(0, ps_s0, m_acc0)

            # ---- range 1 phase 1 interleaved with range 0 phase 2 ----
            m_acc1 = accs.tile([128, FCMAX], DT16, tag="m_acc")
            nc.vector.memset(m_acc1[:], -65504.0)
            ps_s1 = psum_t.tile([1, 512], DT, tag="ps_s1")
            st1 = [0, nmm]
            for ci in range(n_chunks):
                p1_chunk(1, ci, ps_s1, m_acc1, st1)
                p2_chunk(0, ci, mult0)
            mult1 = fold_and_mlp(1, ps_s1, m_acc1)

            # ---- range 1 phase 2: re-read chunks first, stashed chunks
            # last so the kernel tail is mul->write with no read ----
            for ci in range(n_stash_r1, n_chunks):
                p2_chunk(1, ci, mult1)
            for ci in range(n_stash_r1):
                p2_chunk(1, ci, mult1)

    nc.compile()
    return nc


_CACHE: dict[int, object] = {}


def kernel(x, batch_ids, W1, b1, W2, b2):
    x = np.ascontiguousarray(np.asarray(x, dtype=np.float32))
    batch_ids = np.asarray(batch_ids, dtype=np.int32)
    W1 = np.asarray(W1, dtype=np.float32)
    b1 = np.asarray(b1, dtype=np.float32)
    W2 = np.asarray(W2, dtype=np.float32)
    b2 = np.asarray(b2, dtype=np.float32)

    N = x.shape[0]
    bounds = np.searchsorted(batch_ids, np.arange(B + 1), side="left")
    counts = np.diff(bounds)
    R = max(TP, int(-(-counts.max() // TP)) * TP)

    nc = _CACHE.get(R)
    if nc is None:
        nc = _CACHE[R] = build_nc(R)

    xh = x.astype(np.float16)
    xp = np.empty((NCORES, RPC, R, C), np.float16)
    corrt = np.zeros((NCORES, C, RPC), np.float32)
    invct = np.zeros((NCORES, C, RPC), np.float32)
    for b in range(B):
        core, r = divmod(b, RPC)
        s, e = int(bounds[b]), int(bounds[b + 1])
        n = e - s
        xp[core, r, :n] = xh[s:e]
        pad = xh[s] if n > 0 else np.zeros(C, np.float16)
        xp[core, r, n:] = pad
        corrt[core, :, r] = np.float64(R - n) * pad.astype(np.float64)
        invct[core, :, r] = 1.0 / max(n, 1)

    w1t = np.ascontiguousarray(W1.T)  # [C, H]
    b1c = np.ascontiguousarray(b1.reshape(H, 1))
    w2t = np.ascontiguousarray(W2.T)  # [H, C]
    b2s = np.ascontiguousarray((2.0 * b2).reshape(C, 1))

    in_maps = [
        {
            "xs": xp[core],
            "corrt": np.ascontiguousarray(corrt[core]),
            "invct": np.ascontiguousarray(invct[core]),
            "w1t": w1t,
            "b1c": b1c,
            "w2t": w2t,
            "b2s": b2s,
        }
        for core in range(NCORES)
    ]

    res = run_bass_kernel_spmd(nc, in_maps, core_ids=list(range(NCORES)))

    out = np.empty((N, C), np.float32)
    for b in range(B):
        core, r = divmod(b, RPC)
        s, e = int(bounds[b]), int(bounds[b + 1])
        out[s:e] = res.results[core]["out"][r, : e - s].astype(np.float32)
    return out
